# revision 6
# baseline (speedup 1.0000x reference)
"""Trainium2 Bass kernel for the ODE-RNN encoder (nn_Encoder_ODE_RNN).

Data-parallel over 8 NeuronCores: 1024 trajectories -> 128 per core.
Each core runs the 128-step scan with 2 pipelined half-batch streams
(64 trajectories each) so that ACT/DVE work of one stream overlaps
TensorE work of the other. Matmul inputs are fp16 (1 cycle/row on the
PE), accumulation and state are fp32.

Self-contained: hardcodes all shapes from the problem spec.
"""

import numpy as np

# Problem dims
N_TRAJ, N_TP, Y_DIM, H_DIM, R_DIM = 1024, 128, 64, 256, 128
N_GRU, N_ODE = 256, 100
NCORES = 8
B = N_TRAJ // NCORES  # 128 trajectories per core
SW = B // 2           # stream width (64)
HT = 128              # h-tile size (H_DIM = 2 h-tiles)

_CACHE: dict = {}


def _pack_wblob(inp, mmdt):
    """Pack all matmul lhsT tiles into one [128, W_TOT] blob; return
    (blob, offsets dict name -> (K, M, col_offset))."""
    W = {k: np.asarray(inp[k], np.float32) for k in inp if k[0] == "W"}
    bias = {k: np.asarray(inp[k], np.float32) for k in inp if k[0] == "b"}

    segs = []  # (name, array [K, M])
    segs.append(("o1k0", W["Wo1"][0:128]))
    segs.append(("o1k1", W["Wo1"][128:256]))
    segs.append(("o2", W["Wo2"]))
    segs.append(("o3", W["Wo3"]))
    wur1 = np.hstack([W["Wu1"], W["Wr1"]])          # [320, 512]
    bur1 = np.hstack([bias["bu1"], bias["br1"]])    # [512]
    segs.append(("ur1k0", wur1[0:128]))
    segs.append(("ur1k1", wur1[128:256]))
    segs.append(("ur1k2", np.vstack([wur1[256:320], bur1[None]])))  # [65, 512]
    segs.append(("n1k0", W["Wn1"][0:128]))
    segs.append(("n1k1", W["Wn1"][128:256]))
    segs.append(("n1k2", np.vstack([W["Wn1"][256:320], bias["bn1"][None]])))
    segs.append(("u2k0", W["Wu2"][0:128]))
    segs.append(("u2k1", W["Wu2"][128:256]))
    segs.append(("r2k0", W["Wr2"][0:128]))
    segs.append(("r2k1", W["Wr2"][128:256]))
    segs.append(("n2k0", W["Wn2"][0:128]))
    segs.append(("n2k1", W["Wn2"][128:256]))
    segs.append(("fck0", W["Wfc"][0:128]))
    segs.append(("fck1", W["Wfc"][128:256]))
    segs.append(("b_u2r2", np.hstack([bias["bu2"], bias["br2"]])[None]))  # [1,512]
    segs.append(("b_n2", bias["bn2"][None]))
    segs.append(("b_o3", bias["bo3"][None]))

    wtot = sum(a.shape[1] for _, a in segs)
    blob = np.zeros((128, wtot), mmdt)
    offs = {}
    col = 0
    for name, a in segs:
        k, m = a.shape
        blob[0:k, col:col + m] = a.astype(mmdt)
        offs[name] = (k, m, col)
        col += m
    return blob, offs


def _build_program(dts, bias_nz, offs, wtot, xtot):
    """Build + compile the Bacc program. dts: tuple of 128 python floats.
    bias_nz: dict of bools for biases applied via ones-matmuls."""
    import concourse.bacc as bacc
    import concourse.mybir as mybir
    from concourse.tile import TileContext

    f32 = mybir.dt.float32
    f16 = mybir.dt.float16
    Tanh = mybir.ActivationFunctionType.Tanh
    Sigmoid = mybir.ActivationFunctionType.Sigmoid
    Ident = mybir.ActivationFunctionType.Identity
    MULT = mybir.AluOpType.mult
    ADD = mybir.AluOpType.add
    SUB = mybir.AluOpType.subtract

    nc = bacc.Bacc("TRN2", target_bir_lowering=False, debug=False)
    wblob_d = nc.declare_dram_parameter("wblob", [128, wtot], f16, isOutput=False)
    xblob_d = nc.declare_dram_parameter("xblob", [65, xtot], f16, isOutput=False)
    bblob_d = nc.declare_dram_parameter("bblob", [128, 3], f32, isOutput=False)
    out_d = nc.declare_dram_parameter("out", [R_DIM, B], f32, isOutput=True)

    need_ones = any(bias_nz.values())

    with TileContext(nc) as tc:
        with (
            tc.tile_pool(name="persist", bufs=1) as pp,
            tc.tile_pool(name="sb", bufs=2) as sb,
            tc.tile_pool(name="psA", bufs=2, space="PSUM") as psA,
            tc.tile_pool(name="psB", bufs=4, space="PSUM") as psB,
        ):
            wsb = pp.tile([128, wtot], f16, tag="wsb")
            nc.sync.dma_start(out=wsb[:], in_=wblob_d[:])
            xsb = pp.tile([65, xtot], f16, tag="xsb")
            nc.sync.dma_start(out=xsb[:], in_=xblob_d[:])
            bsb = pp.tile([128, 3], f32, tag="bsb")
            nc.sync.dma_start(out=bsb[:], in_=bblob_d[:])

            if need_ones:
                ones16 = pp.tile([1, SW], f16, tag="ones")
                nc.vector.memset(ones16[:], 1.0)

            # per-stream state: [128 part(feat within h-tile), 2*SW free]
            # free layout: h-tile 0 at cols 0:SW, h-tile 1 at SW:2*SW
            y32 = [pp.tile([128, 2 * SW], f32, tag=f"y32_{s}", name=f"y32_{s}") for s in (0, 1)]
            y16 = [pp.tile([128, 2 * SW], f16, tag=f"y16_{s}", name=f"y16_{s}") for s in (0, 1)]
            for s in (0, 1):
                nc.vector.memset(y32[s][:], 0.0)
                nc.vector.memset(y16[s][:], 0.0)

            def w(name, mslice=None):
                k, m, col = offs[name]
                if mslice is None:
                    return wsb[0:k, col:col + m]
                lo, hi = mslice
                return wsb[0:k, col + lo:col + hi]

            for t in range(N_TP):
                dt = float(dts[t])
                xcol = t * B

                # ---- ODE layer 1: g1 = tanh(y @ Wo1 + bo1) ----
                ps_g1 = [psA.tile([128, SW], f32, tag="psa", name="psa") for _ in (0, 1)]
                for s in (0, 1):
                    nc.tensor.matmul(ps_g1[s][0:N_ODE, :], w("o1k0"),
                                     y16[s][:, 0:SW], start=True, stop=False)
                    nc.tensor.matmul(ps_g1[s][0:N_ODE, :], w("o1k1"),
                                     y16[s][:, SW:2 * SW], start=False, stop=True)
                g1 = [sb.tile([N_ODE, SW], f16, tag="g1", name="g1") for _ in (0, 1)]
                for s in (0, 1):
                    nc.scalar.activation(g1[s][:], ps_g1[s][0:N_ODE, :], Tanh,
                                         bias=bsb[0:N_ODE, 0:1])

                # ---- ODE layer 2: g2 = tanh(g1 @ Wo2 + bo2) ----
                ps_g2 = [psA.tile([128, SW], f32, tag="psa", name="psa") for _ in (0, 1)]
                for s in (0, 1):
                    nc.tensor.matmul(ps_g2[s][0:N_ODE, :], w("o2"), g1[s][:],
                                     start=True, stop=True)
                g2 = [sb.tile([N_ODE, SW], f16, tag="g2", name="g2") for _ in (0, 1)]
                for s in (0, 1):
                    nc.scalar.activation(g2[s][:], ps_g2[s][0:N_ODE, :], Tanh,
                                         bias=bsb[0:N_ODE, 1:2])

                # ---- ODE layer 3: g3 = g2 @ Wo3 (+ bo3) ----
                ps_g3 = [psB.tile([128, 2 * SW], f32, tag="ps", name="ps") for _ in (0, 1)]
                for s in (0, 1):
                    for m in (0, 1):
                        seg = ps_g3[s][:, m * SW:(m + 1) * SW]
                        nc.tensor.matmul(seg, w("o3", (m * HT, (m + 1) * HT)),
                                         g2[s][:], start=True,
                                         stop=not bias_nz["bo3"])
                        if bias_nz["bo3"]:
                            nc.tensor.matmul(seg, w("b_o3", (m * HT, (m + 1) * HT)),
                                             ones16[:], start=False, stop=True)

                # ---- y_ode = y + dt * g3 ----
                y_o32 = [sb.tile([128, 2 * SW], f32, tag="y_o32", name="y_o32") for _ in (0, 1)]
                y_o16 = [sb.tile([128, 2 * SW], f16, tag="y_o16", name="y_o16") for _ in (0, 1)]
                for s in (0, 1):
                    nc.vector.scalar_tensor_tensor(
                        y_o32[s][:], ps_g3[s][:], dt, y32[s][:],
                        op0=MULT, op1=ADD)
                    nc.vector.tensor_copy(y_o16[s][:], y_o32[s][:])

                # ---- GRU u/r layer 1: tanh(yc @ [Wu1|Wr1] + b) ----
                # psum segs: m=0: u feats 0:128, 1: u 128:256, 2: r 0:128, 3: r 128:256
                ps_ur1 = [psB.tile([128, 4 * SW], f32, tag="ps", name="ps") for _ in (0, 1)]
                for s in (0, 1):
                    xap = xsb[0:65, xcol + s * SW: xcol + (s + 1) * SW]
                    for m in range(4):
                        seg = ps_ur1[s][:, m * SW:(m + 1) * SW]
                        msl = (m * HT, (m + 1) * HT)
                        nc.tensor.matmul(seg, w("ur1k0", msl), y_o16[s][:, 0:SW],
                                         start=True, stop=False)
                        nc.tensor.matmul(seg, w("ur1k1", msl), y_o16[s][:, SW:2 * SW],
                                         start=False, stop=False)
                        nc.tensor.matmul(seg, w("ur1k2", msl), xap,
                                         start=False, stop=True)
                h_ur = [sb.tile([128, 4 * SW], f16, tag="h_ur", name="h_ur") for _ in (0, 1)]
                for s in (0, 1):
                    nc.scalar.activation(h_ur[s][:], ps_ur1[s][:], Tanh)

                # ---- GRU u/r layer 2 + sigmoid ----
                ps_ur2 = [psB.tile([128, 4 * SW], f32, tag="ps", name="ps") for _ in (0, 1)]
                for s in (0, 1):
                    for g, wn in ((0, "u2k"), (1, "r2k")):
                        bz = bias_nz["bu2"] if g == 0 else bias_nz["br2"]
                        for m in (0, 1):
                            seg = ps_ur2[s][:, (g * 2 + m) * SW:(g * 2 + m + 1) * SW]
                            msl = (m * HT, (m + 1) * HT)
                            for k in (0, 1):
                                rhs = h_ur[s][:, (g * 2 + k) * SW:(g * 2 + k + 1) * SW]
                                nc.tensor.matmul(seg, w(wn + str(k), msl), rhs,
                                                 start=(k == 0),
                                                 stop=(k == 1 and not bz))
                            if bz:
                                bsl = (g * 2 * HT + m * HT, g * 2 * HT + (m + 1) * HT)
                                nc.tensor.matmul(seg, w("b_u2r2", bsl), ones16[:],
                                                 start=False, stop=True)
                ur = [sb.tile([128, 4 * SW], f32, tag="ur", name="ur") for _ in (0, 1)]
                for s in (0, 1):
                    nc.scalar.activation(ur[s][:], ps_ur2[s][:], Sigmoid)

                # ---- c = y_ode * r ----
                c16 = [sb.tile([128, 2 * SW], f16, tag="c16", name="c16") for _ in (0, 1)]
                for s in (0, 1):
                    nc.vector.tensor_tensor(c16[s][:], y_o32[s][:],
                                            ur[s][:, 2 * SW:4 * SW], op=MULT)

                # ---- GRU n layer 1: tanh(cc @ Wn1 + bn1) ----
                ps_n1 = [psB.tile([128, 2 * SW], f32, tag="ps", name="ps") for _ in (0, 1)]
                for s in (0, 1):
                    xap = xsb[0:65, xcol + s * SW: xcol + (s + 1) * SW]
                    for m in (0, 1):
                        seg = ps_n1[s][:, m * SW:(m + 1) * SW]
                        msl = (m * HT, (m + 1) * HT)
                        nc.tensor.matmul(seg, w("n1k0", msl), c16[s][:, 0:SW],
                                         start=True, stop=False)
                        nc.tensor.matmul(seg, w("n1k1", msl), c16[s][:, SW:2 * SW],
                                         start=False, stop=False)
                        nc.tensor.matmul(seg, w("n1k2", msl), xap,
                                         start=False, stop=True)
                n1 = [sb.tile([128, 2 * SW], f16, tag="n1", name="n1") for _ in (0, 1)]
                for s in (0, 1):
                    nc.scalar.activation(n1[s][:], ps_n1[s][:], Tanh)

                # ---- GRU n layer 2 (linear) ----
                ps_n2 = [psB.tile([128, 2 * SW], f32, tag="ps", name="ps") for _ in (0, 1)]
                for s in (0, 1):
                    for m in (0, 1):
                        seg = ps_n2[s][:, m * SW:(m + 1) * SW]
                        msl = (m * HT, (m + 1) * HT)
                        for k in (0, 1):
                            nc.tensor.matmul(seg, w("n2k" + str(k), msl),
                                             n1[s][:, k * SW:(k + 1) * SW],
                                             start=(k == 0),
                                             stop=(k == 1 and not bias_nz["bn2"]))
                        if bias_nz["bn2"]:
                            nc.tensor.matmul(seg, w("b_n2", msl), ones16[:],
                                             start=False, stop=True)

                # ---- blend: y = u*y_ode - (u-1)*n ----
                v32 = [sb.tile([128, 2 * SW], f32, tag="v32", name="v32") for _ in (0, 1)]
                t32 = [sb.tile([128, 2 * SW], f32, tag="t32", name="t32") for _ in (0, 1)]
                for s in (0, 1):
                    nc.vector.scalar_tensor_tensor(
                        v32[s][:], ur[s][:, 0:2 * SW], 1.0, ps_n2[s][:],
                        op0=SUB, op1=MULT)
                    nc.vector.tensor_tensor(t32[s][:], ur[s][:, 0:2 * SW],
                                            y_o32[s][:], op=MULT)
                    nc.vector.tensor_tensor(y32[s][:], t32[s][:], v32[s][:], op=SUB)
                    nc.vector.tensor_copy(y16[s][:], y32[s][:])

            # ---- output head: out = y @ Wfc + bfc, [R_DIM, B] ----
            outsb = pp.tile([R_DIM, B], f32, tag="outsb")
            for s in (0, 1):
                ps_fc = psA.tile([128, SW], f32, tag="psa", name="ps_fc")
                nc.tensor.matmul(ps_fc[:], w("fck0"), y16[s][:, 0:SW],
                                 start=True, stop=False)
                nc.tensor.matmul(ps_fc[:], w("fck1"), y16[s][:, SW:2 * SW],
                                 start=False, stop=True)
                nc.scalar.activation(outsb[:, s * SW:(s + 1) * SW], ps_fc[:],
                                     Ident, bias=bsb[:, 2:3])
            nc.sync.dma_start(out=out_d[:], in_=outsb[:])

    nc.compile()
    return nc


def _make_runner(nc):
    """Build a reusable jitted 8-core runner (mirrors
    bass2jax.run_bass_via_pjrt but caches the jitted callable)."""
    import jax
    import numpy as _np
    import concourse.mybir as mybir
    from concourse import bass2jax
    from jax.sharding import Mesh, PartitionSpec
    from jax.experimental.shard_map import shard_map

    bass2jax.install_neuronx_cc_hook()

    partition_name = nc.partition_id_tensor.name if nc.partition_id_tensor else None

    in_names, out_names, out_avals, zero_outs = [], [], [], []
    for alloc in nc.m.functions[0].allocations:
        if not isinstance(alloc, mybir.MemoryLocationSet):
            continue
        name = alloc.memorylocations[0].name
        if alloc.kind == "ExternalInput":
            if name != partition_name:
                in_names.append(name)
        elif alloc.kind == "ExternalOutput":
            shape = tuple(alloc.tensor_shape)
            dtype = mybir.dt.np(alloc.dtype)
            out_names.append(name)
            out_avals.append(jax.core.ShapedArray(shape, dtype))
            zero_outs.append(_np.zeros(shape, dtype))
    n_params = len(in_names)
    n_outs = len(out_avals)
    all_names = in_names + out_names
    if partition_name is not None:
        all_names = all_names + [partition_name]

    def _body(*args):
        operands = list(args)
        if partition_name is not None:
            operands.append(bass2jax.partition_id_tensor())
        outs = bass2jax._bass_exec_p.bind(
            *operands,
            out_avals=tuple(out_avals),
            in_names=tuple(all_names),
            out_names=tuple(out_names),
            lowering_input_output_aliases=(),
            sim_require_finite=True,
            sim_require_nnan=True,
            nc=nc,
        )
        return tuple(outs)

    devices = jax.devices()[:NCORES]
    mesh = Mesh(_np.asarray(devices), ("core",))
    in_specs = (PartitionSpec("core"),) * (n_params + n_outs)
    out_specs = (PartitionSpec("core"),) * n_outs
    sharded = jax.jit(
        shard_map(_body, mesh=mesh, in_specs=in_specs, out_specs=out_specs,
                  check_rep=False),
        keep_unused=True,
    )

    def run(in_maps):
        concat_in = [
            _np.concatenate([_np.asarray(in_maps[c][n]) for c in range(NCORES)], 0)
            for n in in_names
        ]
        concat_zero = [
            _np.zeros((NCORES * z.shape[0], *z.shape[1:]), z.dtype)
            for z in zero_outs
        ]
        out_arrs = sharded(*concat_in, *concat_zero)
        return [
            {n: _np.asarray(out_arrs[i]).reshape(NCORES, *out_avals[i].shape)[c]
             for i, n in enumerate(out_names)}
            for c in range(NCORES)
        ]

    return run


def _prepare(inputs):
    mmdt = np.float16
    data = np.asarray(inputs["data"], np.float32)
    ts = np.asarray(inputs["time_steps"], np.float32)[0]
    dts = np.concatenate([np.full((1,), -0.01, np.float32),
                          (ts[:-1] - ts[1:])[::-1]]).astype(np.float32)

    wblob, offs = _pack_wblob(inputs, mmdt)

    bias_nz = {}
    for bn in ("bu2", "br2", "bn2", "bo3"):
        bias_nz[bn] = bool(np.any(np.asarray(inputs[bn], np.float32) != 0.0))

    bblob = np.zeros((128, 3), np.float32)
    bblob[0:N_ODE, 0] = np.asarray(inputs["bo1"], np.float32)
    bblob[0:N_ODE, 1] = np.asarray(inputs["bo2"], np.float32)
    bblob[0:R_DIM, 2] = np.asarray(inputs["bfc"], np.float32)

    # x blobs: [65, N_TP*B] per core; row 64 = ones (bias row for layer-1 nets)
    xs_rev = data[:, ::-1, :]  # [1024, 128, 64] time-reversed
    xblobs = []
    for c in range(NCORES):
        shard = xs_rev[c * B:(c + 1) * B]          # [B, NT, Y]
        xb = np.empty((65, N_TP * B), mmdt)
        xb[0:Y_DIM] = shard.transpose(2, 1, 0).reshape(Y_DIM, N_TP * B).astype(mmdt)
        xb[Y_DIM] = np.ones(N_TP * B, mmdt)
        xblobs.append(xb)

    return dts, bias_nz, wblob, offs, bblob, xblobs


def kernel(**inputs) -> np.ndarray:
    dts, bias_nz, wblob, offs, bblob, xblobs = _prepare(inputs)

    key = (dts.tobytes(), tuple(sorted(bias_nz.items())), wblob.shape[1])
    if key not in _CACHE:
        nc = _build_program(tuple(float(d) for d in dts), bias_nz, offs,
                            wblob.shape[1], N_TP * B)
        _CACHE[key] = _make_runner(nc)
    run = _CACHE[key]

    in_maps = [
        {"wblob": wblob, "xblob": xblobs[c], "bblob": bblob}
        for c in range(NCORES)
    ]
    results = run(in_maps)

    out = np.empty((N_TRAJ, R_DIM), np.float32)
    for c in range(NCORES):
        out[c * B:(c + 1) * B] = results[c]["out"].T  # [R,B] -> [B,R]
    return out
